# revision 6
# baseline (speedup 1.0000x reference)
"""Trainium2 Bass kernel: NKQuantizer2 top-8 masking (k=8). ~196us HW.

reference:  kh = topk_hot(x, 8); out = einsum('bsq,eq->bse', kh, W)

Per 128-token tile (algorithm "B" on all tiles, n_b_tiles=8 default):
  1. max8(x [128,8192] f32)        -> exact top-8 values   (~8.6us DVE)
  2. find_index8(x, v8)            -> their q indices      (~8.6us DVE)
     (find_index8 maps duplicate needles to successive occurrences,
      matching jax top_k's tie-by-index behavior; ~0.35% of tokens have
      exact duplicate values in their top-8 with this data)
  3. 8 indirect W-row gathers bf16 ([128,1]-offset each: batched
     multi-offset APs are silently broken on real HW)
  4. DVE tree reduce (bf16 2x, final level writes f32)
  5. f32 store on the ACT HWDGE ring (own ring; SP ring holds x loads --
     a ring holds 16 in-flight DMAs before ring-credit waits appear)

An alternative hierarchical path (chunk-max + chunk re-gather, one full
DVE pass instead of two; n_b_tiles<8 enables it) measures WORSE end to
end (243us pure): its ~2us/tile DVE saving is outweighed by +9us/tile
GpSimd descriptor generation (INDIRECT1D = ~1.1us per 128 descriptors),
HBM-inefficient 256B chunk reads, small-op overhead (~200-255ns per tiny
DVE instr), and SWDGE semaphore-lane (8 lanes) recycling stalls.

Pipeline: software-pipelined over 8 tiles; DVE stream [s1_i | s2_{i-1} |
s3_{i-4}] so the last tiles' W-gather latency hides behind other tiles'
tree-reduces; xt rotation depth 3, gw/out depth 5.

Sync discipline (hard-won, all verified on HW):
  - toolchain allows at most ONE semaphore wait per instruction, incl.
    DMAs (whose only slot the HWDGE ring-credit wait may occupy) and the
    TileContext exit drain (quiesced via one single-wait SP NOP per DMA);
  - same-engine program order does NOT protect read-after-write through
    SBUF (the write-ack races the next instruction's read: max8 ->
    find_index8 back-to-back intermittently returns all-0xFFFFFFFF
    "not found"); every DVE->DVE RAW edge must be a real semaphore dep
    (the engine sem increments on write completion);
  - instructions whose inputs are all DMA-written carry the DMA
    completion wait instead and ride queue order w.r.t. the DVE stream;
  - multi-dependency consumers split their waits across single-wait
    NOPs on their own queue.
"""

import numpy as np
import ml_dtypes

import concourse.bass as bass
import concourse.mybir as mybir
import concourse.tile as tile
from concourse.bass_utils import run_bass_kernel_spmd
from concourse.tile_rust import add_dep_helper

B, S, Q, E, TOPK = 4, 2048, 8192, 512, 8
N_CORES = 8
P = 128
CW = 64                      # chunk width
NCH = Q // CW                # 128 chunks per token
T_TOTAL = B * S              # 8192 tokens
T_CORE = T_TOTAL // N_CORES  # 1024 tokens per core

F32 = mybir.dt.float32
BF16 = mybir.dt.bfloat16
U32 = mybir.dt.uint32

Alu = mybir.AluOpType


import os


def build_bass(t_core=T_CORE, debug=False, n_b_tiles=None):
    if n_b_tiles is None:
        n_b_tiles = int(os.environ.get("NB", "8"))
    n_tiles = t_core // P
    XB, GB, WB, OB = 4, 3, 5, 5   # rotation depths
    # algo B (two full DVE passes, no chunk re-gather) on a few tiles
    # balances POOL (descriptor-gen-bound) vs DVE across the core
    if n_b_tiles == 0:
        is_b = [False] * n_tiles
    else:
        step = n_tiles / n_b_tiles
        bset = {int((k + 0.5) * step) for k in range(n_b_tiles)}
        is_b = [i in bset for i in range(n_tiles)]

    nc = bass.Bass(trn_type="TRN2", target_bir_lowering=False)
    # x viewed as chunk rows [t_core*128, 64]
    xd = nc.dram_tensor("x", [t_core * NCH, CW], F32, kind="ExternalInput")
    wt = nc.dram_tensor("wt", [Q, E], BF16, kind="ExternalInput")
    out_d = nc.dram_tensor("out", [t_core, E], F32, kind="ExternalOutput")
    if debug:
        dbg_c = nc.dram_tensor("dbg_c", [t_core, NCH], F32, kind="ExternalOutput")
        dbg_ci8 = nc.dram_tensor("dbg_ci8", [t_core, 8], U32, kind="ExternalOutput")
        dbg_row8 = nc.dram_tensor("dbg_row8", [t_core, 8], U32, kind="ExternalOutput")
        dbg_q8 = nc.dram_tensor("dbg_q8", [t_core, 8], U32, kind="ExternalOutput")
        dbg_v8 = nc.dram_tensor("dbg_v8", [t_core, 8], F32, kind="ExternalOutput")
        dbg_g = nc.dram_tensor("dbg_g", [t_core, TOPK, CW], F32, kind="ExternalOutput")

    dve_q, pool_q, sp_q, act_q = [], [], [], []
    dmas = []

    def strip(bi):
        try:
            names = list(bi.ins.sync_dependency_names())
        except TypeError:
            names = list(bi.ins.sync_dependency_names)
        for n in names:
            bi.ins.try_remove_dependency(n)
        return bi

    def q(queue, bi, dep=None):
        """Strip auto deps; nosync-chain on queue; optionally ONE sync dep."""
        strip(bi)
        if queue:
            add_dep_helper(bi.ins, queue[-1].ins, False, "queue order")
        if dep is not None:
            add_dep_helper(bi.ins, dep.ins, True, "data dep")
        queue.append(bi)
        return bi

    def dve(bi, dep=None):
        """DVE stream discipline: same-engine RAW through SBUF is NOT
        protected by program order (write-ack races the next read - the
        TRN2 SBUF read-write bubble). An instruction either carries its
        cross-engine/DMA wait (inputs DMA-written, no DVE RAW), or it
        waits its DVE predecessor's completion semaphore (ack-backed)."""
        strip(bi)
        if dep is not None:
            if dve_q:
                add_dep_helper(bi.ins, dve_q[-1].ins, False, "queue order")
            add_dep_helper(bi.ins, dep.ins, True, "data dep")
        elif dve_q:
            add_dep_helper(bi.ins, dve_q[-1].ins, True, "dve raw chain")
        dve_q.append(bi)
        return bi

    def pool(bi, dep=None):
        return q(pool_q, bi, dep)

    def sp(bi, dep=None):
        return q(sp_q, bi, dep)

    def act(bi, dep=None):
        return q(act_q, bi, dep)

    with tile.TileContext(nc) as tc:
        with (
            tc.tile_pool(name="xpool", bufs=XB) as xpool,
            tc.tile_pool(name="gpool", bufs=GB) as gpool,
            tc.tile_pool(name="wpool", bufs=WB) as wpool,
            tc.tile_pool(name="opool", bufs=OB) as opool,
            tc.tile_pool(name="cpool", bufs=1) as cpool,
            tc.tile_pool(name="spool", bufs=1) as spool,
        ):
            xts = [xpool.tile([P, Q], F32, name=f"xt{i}", tag="xt") for i in range(XB)]
            gs = [gpool.tile([P, TOPK, CW], F32, name=f"g{i}", tag="g") for i in range(GB)]
            gws = [wpool.tile([P, TOPK, E], BF16, name=f"gw{i}", tag="gw") for i in range(WB)]
            outfs = [opool.tile([P, E], F32, name=f"of{i}", tag="of") for i in range(OB)]
            iota_u = cpool.tile([P, 1], U32, name="iota_u")
            iota_f = cpool.tile([P, 1], F32, name="iota_f")

            # prologue: per-partition p*128 constant
            i_iota = pool(nc.gpsimd.iota(iota_u[:], [[1, 1]], channel_multiplier=P))
            dve(nc.vector.tensor_copy(iota_f[:], iota_u[:]), dep=i_iota)

            # per-tile state
            st = [dict() for _ in range(n_tiles)]

            def s_tile(i, shape, dt, nm):
                return spool.tile(shape, dt, name=f"{nm}{i}")

            def emit_xload(i):
                # two half-row loads so the first max8 half can start as
                # soon as 2MB (not 4MB) has landed -- trims the ramp
                xt = xts[i % XB]
                src_full = xd[i * P * NCH : (i + 1) * P * NCH, :].rearrange(
                    "(t c) w -> t (c w)", c=NCH
                )
                src_a = src_full[:, : Q // 2]
                src_b = src_full[:, Q // 2 :]
                if 2 <= i < XB:
                    first = st[i - 2].get("m8") or st[i - 2].get("red")
                    if first is not None:
                        sp(nc.sync.nop(), dep=first)
                if i >= XB:
                    # WAR on xt rotation rides a gating NOP (HWDGE DMAs
                    # cannot carry manual waits: the ring-credit wait
                    # occupies their single wait slot)
                    sp(nc.sync.nop(), dep=st[i - XB]["xt_done"])
                da = sp(nc.sync.dma_start(xt[:, : Q // 2], src_a))
                db = sp(nc.sync.dma_start(xt[:, Q // 2 :], src_b))
                dmas.append(da)
                dmas.append(db)
                st[i]["xload_a"] = da
                st[i]["xload"] = db

            def emit_s1_b(i):
                """algo B stage 1: exact top-8 values, two halves + merge
                (the merged top-8 of per-half top-8s equals the full top-8;
                halves let compute start after the first half-load)."""
                xt = xts[i % XB]
                ab = s_tile(i, [P, 16], F32, "ab")
                v8 = s_tile(i, [P, 8], F32, "v8")
                dve(
                    nc.vector.max(out=ab[:, 0:8], in_=xt[:, : Q // 2]),
                    dep=st[i]["xload_a"],
                )
                # ack-fence: guarantee the first half's write landed before
                # the merge (the second max8 only rides queue order)
                dve(nc.vector.nop())
                dve(
                    nc.vector.max(out=ab[:, 8:16], in_=xt[:, Q // 2 :]),
                    dep=st[i]["xload"],
                )
                st[i]["m8"] = dve(nc.vector.max(out=v8[:], in_=ab[:]))
                st[i]["v8_tile"] = v8

            def emit_s2_b(i):
                """algo B stage 2: indices via full-row find_index8."""
                xt = xts[i % XB]
                q8u = s_tile(i, [P, 8], U32, "q8u")
                st[i]["q8"] = dve(nc.vector.max_index(
                    out=q8u[:], in_max=st[i]["v8_tile"][:], in_values=xt[:],
                ))
                st[i]["xt_done"] = st[i]["q8"]
                st[i]["q8_tile"] = q8u

            def emit_s1(i):
                """chunk-max + chunk selection + gather-row math."""
                xt = xts[i % XB]
                c = s_tile(i, [P, NCH], F32, "c")
                cv8 = s_tile(i, [P, 8], F32, "cv8")
                ci8 = s_tile(i, [P, 8], U32, "ci8")
                ci8f = s_tile(i, [P, 8], F32, "ci8f")
                ci64f = s_tile(i, [P, 8], F32, "ci64f")
                row8f = s_tile(i, [P, 8], F32, "row8f")
                row8 = s_tile(i, [P, 8], U32, "row8")
                st[i]["red"] = dve(
                    nc.vector.tensor_reduce(
                        out=c[:],
                        in_=xt[:].rearrange("t (c w) -> t c w", w=CW),
                        axis=mybir.AxisListType.X,
                        op=Alu.max,
                    ),
                    dep=st[i]["xload"],
                )
                st[i]["xt_done"] = st[i]["red"]
                dve(nc.vector.max(out=cv8[:], in_=c[:]))
                dve(nc.vector.max_index(out=ci8[:], in_max=cv8[:], in_values=c[:]))
                dve(nc.vector.tensor_copy(ci8f[:], ci8[:]))
                dve(nc.vector.tensor_scalar(
                    out=ci64f[:], in0=ci8f[:], scalar1=64.0, scalar2=None,
                    op0=Alu.mult,
                ))
                dve(nc.vector.tensor_scalar(
                    out=row8f[:], in0=ci8f[:], scalar1=iota_f[:, 0:1],
                    scalar2=float(i * P * NCH), op0=Alu.add, op1=Alu.add,
                ))
                dve(nc.vector.tensor_scalar(
                    out=row8f[:], in0=row8f[:],
                    scalar1=float(t_core * NCH - 1), scalar2=0.0,
                    op0=Alu.min, op1=Alu.max,
                ))
                st[i]["row8"] = dve(nc.vector.tensor_copy(row8[:], row8f[:]))
                st[i]["row8_tile"] = row8
                st[i]["ci64f"] = ci64f
                if debug:
                    sl = slice(i * P, (i + 1) * P)
                    sp(nc.sync.nop(), dep=st[i]["row8"])
                    dmas.append(sp(nc.sync.dma_start(dbg_c[sl, :], c[:])))
                    dmas.append(sp(nc.sync.dma_start(dbg_ci8[sl, :], ci8[:])))
                    dmas.append(sp(nc.sync.dma_start(dbg_row8[sl, :], row8[:])))

            g_last_reader = {}   # buffer slot -> last instr reading it

            def emit_cgathers(i):
                g = gs[i % GB]
                pool(nc.gpsimd.nop(), dep=st[i]["row8"])
                prev_rd = g_last_reader.get(i % GB)
                if prev_rd is not None:
                    pool(nc.gpsimd.nop(), dep=prev_rd)
                cg = []
                r8 = st[i]["row8_tile"]
                for j in range(TOPK):
                    d = pool(nc.gpsimd.indirect_dma_start(
                        out=g[:, j, :], out_offset=None,
                        in_=xd[:],
                        in_offset=bass.IndirectOffsetOnAxis(
                            ap=r8[:, j : j + 1], axis=0
                        ),
                        compute_op=Alu.bypass,
                    ))
                    dmas.append(d)
                    cg.append(d)
                st[i]["cg"] = cg

            def emit_s2(i):
                """exact values + global index reconstruction."""
                g = gs[i % GB]
                v8 = s_tile(i, [P, 8], F32, "v8")
                local = s_tile(i, [P, 8], U32, "local")
                slot = s_tile(i, [P, 8], U32, "slot")
                slotf = s_tile(i, [P, 8], F32, "slotf")
                within = s_tile(i, [P, 8], U32, "within")
                withinf = s_tile(i, [P, 8], F32, "withinf")
                sel3 = s_tile(i, [P, 8, 8], F32, "sel3")
                csel = s_tile(i, [P, 8], F32, "csel")
                q8f = s_tile(i, [P, 8], F32, "q8f")
                q8u = s_tile(i, [P, 8], U32, "q8u")
                cg = st[i]["cg"]
                for j in range(TOPK - 1):
                    dve(nc.vector.nop(), dep=cg[j])
                dve(nc.vector.max(out=v8[:], in_=g[:]), dep=cg[TOPK - 1])
                st[i]["fi8g"] = dve(nc.vector.max_index(
                    out=local[:], in_max=v8[:],
                    in_values=g[:].rearrange("t s w -> t (s w)"),
                ))
                g_last_reader[i % GB] = st[i]["fi8g"]
                dve(nc.vector.tensor_scalar(
                    out=slot[:], in0=local[:], scalar1=6, scalar2=None,
                    op0=Alu.logical_shift_right,
                ))
                dve(nc.vector.tensor_scalar(
                    out=within[:], in0=local[:], scalar1=63, scalar2=None,
                    op0=Alu.bitwise_and,
                ))
                dve(nc.vector.tensor_copy(slotf[:], slot[:]))
                dve(nc.vector.tensor_copy(withinf[:], within[:]))
                ci64f = st[i]["ci64f"]
                for s in range(8):
                    dve(nc.vector.tensor_scalar(
                        out=sel3[:, :, s], in0=slotf[:], scalar1=float(s),
                        scalar2=ci64f[:, s : s + 1],
                        op0=Alu.is_equal, op1=Alu.mult,
                    ))
                dve(nc.vector.tensor_reduce(
                    out=csel[:], in_=sel3[:], axis=mybir.AxisListType.X,
                    op=Alu.add,
                ))
                dve(nc.vector.tensor_tensor(q8f[:], csel[:], withinf[:], Alu.add))
                dve(nc.vector.tensor_scalar(
                    out=q8f[:], in0=q8f[:],
                    scalar1=float(Q - 1), scalar2=0.0,
                    op0=Alu.min, op1=Alu.max,
                ))
                st[i]["q8"] = dve(nc.vector.tensor_copy(q8u[:], q8f[:]))
                st[i]["q8_tile"] = q8u
                if debug:
                    sl = slice(i * P, (i + 1) * P)
                    sp(nc.sync.nop(), dep=st[i]["q8"])
                    dmas.append(sp(nc.sync.dma_start(dbg_q8[sl, :], q8u[:])))
                    dmas.append(sp(nc.sync.dma_start(dbg_v8[sl, :], v8[:])))
                    dmas.append(sp(nc.sync.dma_start(dbg_g[sl, :, :], g[:])))

            gw_last_reader = {}

            def emit_wgathers(i):
                gw = gws[i % WB]
                pool(nc.gpsimd.nop(), dep=st[i]["q8"])
                prev_rd = gw_last_reader.get(i % WB)
                if prev_rd is not None:
                    pool(nc.gpsimd.nop(), dep=prev_rd)
                q8u = st[i]["q8_tile"]
                wg = []
                for j in range(TOPK):
                    d = pool(nc.gpsimd.indirect_dma_start(
                        out=gw[:, j, :], out_offset=None,
                        in_=wt[:],
                        in_offset=bass.IndirectOffsetOnAxis(
                            ap=q8u[:, j : j + 1], axis=0
                        ),
                        compute_op=Alu.bypass,
                    ))
                    dmas.append(d)
                    wg.append(d)
                st[i]["wg"] = wg

            def emit_s3(i):
                gw = gws[i % WB]
                outf = outfs[i % OB]
                wg = st[i]["wg"]
                for j in range(TOPK - 1):
                    dve(nc.vector.nop(), dep=wg[j])
                if i >= OB:
                    dve(nc.vector.nop(), dep=st[i - OB]["ostore"])
                dve(nc.vector.tensor_tensor(
                    gw[:, 0:4, :], gw[:, 0:4, :], gw[:, 4:8, :], Alu.add
                ), dep=wg[TOPK - 1])
                dve(nc.vector.tensor_tensor(
                    gw[:, 0:2, :], gw[:, 0:2, :], gw[:, 2:4, :], Alu.add
                ))
                st[i]["a3"] = dve(nc.vector.tensor_tensor(
                    outf[:], gw[:, 0, :], gw[:, 1, :], Alu.add
                ))
                gw_last_reader[i % WB] = st[i]["a3"]

            def emit_ostore(i):
                outf = outfs[i % OB]
                act(nc.scalar.nop(), dep=st[i]["a3"])
                d = act(nc.scalar.dma_start(out_d[i * P : (i + 1) * P, :], outf[:]))
                dmas.append(d)
                st[i]["ostore"] = d

            # software pipeline:
            #   DVE:  S1_i | S2_{i-1} | S3_{i-2}
            #   POOL: CG_i (after row8_i) | WG_{i-1} (after q8_{i-1})
            #   SP:   xload_{i+2} | ostore_{i-2}
            # prologue loads
            for i in range(min(2, n_tiles)):
                emit_xload(i)

            S2D = int(os.environ.get("S2D", "1"))
            for i in range(n_tiles + 4):
                if i < n_tiles:
                    if is_b[i]:
                        emit_s1_b(i)
                    else:
                        emit_s1(i)
                        emit_cgathers(i)
                if 0 <= i - S2D < n_tiles:
                    if is_b[i - S2D]:
                        emit_s2_b(i - S2D)
                    else:
                        emit_s2(i - S2D)
                    emit_wgathers(i - S2D)
                if i + 2 < n_tiles:
                    emit_xload(i + 2)
                if 0 <= i - 4 < n_tiles:
                    emit_s3(i - 4)
                    emit_ostore(i - 4)

            # tail quiesce
            for d in dmas:
                n = nc.sync.nop()
                strip(n)
                add_dep_helper(n.ins, d.ins, True, "tail quiesce")
            for lastq in (dve_q, pool_q):
                n = nc.sync.nop()
                strip(n)
                add_dep_helper(n.ins, lastq[-1].ins, True, "tail quiesce")

    return nc


def _prep_wt(W: np.ndarray) -> np.ndarray:
    return np.ascontiguousarray(W.T).astype(ml_dtypes.bfloat16)


_CACHED = {}


def _get_nc():
    if "nc" not in _CACHED:
        _CACHED["nc"] = build_bass()
    return _CACHED["nc"]


def kernel(x: np.ndarray, W: np.ndarray) -> np.ndarray:
    x = np.asarray(x, dtype=np.float32)
    W = np.asarray(W, dtype=np.float32)
    assert x.shape == (B, S, Q) and W.shape == (E, Q)

    nc = _get_nc()
    xf = x.reshape(T_TOTAL, Q)
    WT = _prep_wt(W)
    in_maps = [
        {
            "x": np.ascontiguousarray(
                xf[c * T_CORE : (c + 1) * T_CORE]
            ).reshape(T_CORE * NCH, CW),
            "wt": WT,
        }
        for c in range(N_CORES)
    ]
    res = run_bass_kernel_spmd(nc, in_maps, core_ids=list(range(N_CORES)))
    out = np.concatenate([r["out"] for r in res.results], axis=0)
    return np.ascontiguousarray(out.reshape(B, S, E).astype(np.float32))


# revision 7
# speedup vs baseline: 1.0109x; 1.0109x over previous
"""Trainium2 Bass kernel: NKQuantizer2 top-8 masking (k=8). ~196us HW.

reference:  kh = topk_hot(x, 8); out = einsum('bsq,eq->bse', kh, W)

Per 128-token tile (algorithm "B" on all tiles, n_b_tiles=8 default):
  1. max8(x [128,8192] f32)        -> exact top-8 values   (~8.6us DVE)
  2. find_index8(x, v8)            -> their q indices      (~8.6us DVE)
     (find_index8 maps duplicate needles to successive occurrences,
      matching jax top_k's tie-by-index behavior; ~0.35% of tokens have
      exact duplicate values in their top-8 with this data)
  3. 8 indirect W-row gathers bf16 ([128,1]-offset each: batched
     multi-offset APs are silently broken on real HW)
  4. DVE tree reduce (bf16 2x, final level writes f32)
  5. f32 store on the ACT HWDGE ring (own ring; SP ring holds x loads --
     a ring holds 16 in-flight DMAs before ring-credit waits appear)

An alternative hierarchical path (chunk-max + chunk re-gather, one full
DVE pass instead of two; n_b_tiles<8 enables it) measures WORSE end to
end (243us pure): its ~2us/tile DVE saving is outweighed by +9us/tile
GpSimd descriptor generation (INDIRECT1D = ~1.1us per 128 descriptors),
HBM-inefficient 256B chunk reads, small-op overhead (~200-255ns per tiny
DVE instr), and SWDGE semaphore-lane (8 lanes) recycling stalls.

Pipeline: software-pipelined over 8 tiles; DVE stream [s1_i | s2_{i-1} |
s3_{i-4}] so the last tiles' W-gather latency hides behind other tiles'
tree-reduces; xt rotation depth 3, gw/out depth 5.

Sync discipline (hard-won, all verified on HW):
  - toolchain allows at most ONE semaphore wait per instruction, incl.
    DMAs (whose only slot the HWDGE ring-credit wait may occupy) and the
    TileContext exit drain (quiesced via one single-wait SP NOP per DMA);
  - same-engine program order does NOT protect read-after-write through
    SBUF (the write-ack races the next instruction's read: max8 ->
    find_index8 back-to-back intermittently returns all-0xFFFFFFFF
    "not found"); every DVE->DVE RAW edge must be a real semaphore dep
    (the engine sem increments on write completion);
  - instructions whose inputs are all DMA-written carry the DMA
    completion wait instead and ride queue order w.r.t. the DVE stream;
  - multi-dependency consumers split their waits across single-wait
    NOPs on their own queue.
"""

import numpy as np
import ml_dtypes

import concourse.bass as bass
import concourse.mybir as mybir
import concourse.tile as tile
from concourse.bass_utils import run_bass_kernel_spmd
from concourse.tile_rust import add_dep_helper

B, S, Q, E, TOPK = 4, 2048, 8192, 512, 8
N_CORES = 8
P = 128
CW = 64                      # chunk width
NCH = Q // CW                # 128 chunks per token
T_TOTAL = B * S              # 8192 tokens
T_CORE = T_TOTAL // N_CORES  # 1024 tokens per core

F32 = mybir.dt.float32
BF16 = mybir.dt.bfloat16
U32 = mybir.dt.uint32

Alu = mybir.AluOpType


import os


def build_bass(t_core=T_CORE, debug=False, n_b_tiles=None):
    if n_b_tiles is None:
        n_b_tiles = int(os.environ.get("NB", "8"))
    n_tiles = t_core // P
    XB, GB, WB, OB = 4, 3, 5, 5   # rotation depths
    # algo B (two full DVE passes, no chunk re-gather) on a few tiles
    # balances POOL (descriptor-gen-bound) vs DVE across the core
    if n_b_tiles == 0:
        is_b = [False] * n_tiles
    else:
        step = n_tiles / n_b_tiles
        bset = {int((k + 0.5) * step) for k in range(n_b_tiles)}
        is_b = [i in bset for i in range(n_tiles)]

    nc = bass.Bass(trn_type="TRN2", target_bir_lowering=False)
    # x viewed as chunk rows [t_core*128, 64]
    xd = nc.dram_tensor("x", [t_core * NCH, CW], F32, kind="ExternalInput")
    wt = nc.dram_tensor("wt", [Q, E], BF16, kind="ExternalInput")
    out_d = nc.dram_tensor("out", [t_core, E], F32, kind="ExternalOutput")
    if debug:
        dbg_c = nc.dram_tensor("dbg_c", [t_core, NCH], F32, kind="ExternalOutput")
        dbg_ci8 = nc.dram_tensor("dbg_ci8", [t_core, 8], U32, kind="ExternalOutput")
        dbg_row8 = nc.dram_tensor("dbg_row8", [t_core, 8], U32, kind="ExternalOutput")
        dbg_q8 = nc.dram_tensor("dbg_q8", [t_core, 8], U32, kind="ExternalOutput")
        dbg_v8 = nc.dram_tensor("dbg_v8", [t_core, 8], F32, kind="ExternalOutput")
        dbg_g = nc.dram_tensor("dbg_g", [t_core, TOPK, CW], F32, kind="ExternalOutput")

    dve_q, pool_q, sp_q, act_q = [], [], [], []
    dmas = []

    def strip(bi):
        try:
            names = list(bi.ins.sync_dependency_names())
        except TypeError:
            names = list(bi.ins.sync_dependency_names)
        for n in names:
            bi.ins.try_remove_dependency(n)
        return bi

    def q(queue, bi, dep=None):
        """Strip auto deps; nosync-chain on queue; optionally ONE sync dep."""
        strip(bi)
        if queue:
            add_dep_helper(bi.ins, queue[-1].ins, False, "queue order")
        if dep is not None:
            add_dep_helper(bi.ins, dep.ins, True, "data dep")
        queue.append(bi)
        return bi

    def dve(bi, dep=None):
        """DVE stream discipline: same-engine RAW through SBUF is NOT
        protected by program order (write-ack races the next read - the
        TRN2 SBUF read-write bubble). An instruction either carries its
        cross-engine/DMA wait (inputs DMA-written, no DVE RAW), or it
        waits its DVE predecessor's completion semaphore (ack-backed)."""
        strip(bi)
        if dep is not None:
            if dve_q:
                add_dep_helper(bi.ins, dve_q[-1].ins, False, "queue order")
            add_dep_helper(bi.ins, dep.ins, True, "data dep")
        elif dve_q:
            add_dep_helper(bi.ins, dve_q[-1].ins, True, "dve raw chain")
        dve_q.append(bi)
        return bi

    def pool(bi, dep=None):
        return q(pool_q, bi, dep)

    def sp(bi, dep=None):
        return q(sp_q, bi, dep)

    def act(bi, dep=None):
        return q(act_q, bi, dep)

    with tile.TileContext(nc) as tc:
        with (
            tc.tile_pool(name="xpool", bufs=XB) as xpool,
            tc.tile_pool(name="gpool", bufs=GB) as gpool,
            tc.tile_pool(name="wpool", bufs=WB) as wpool,
            tc.tile_pool(name="opool", bufs=OB) as opool,
            tc.tile_pool(name="cpool", bufs=1) as cpool,
            tc.tile_pool(name="spool", bufs=1) as spool,
        ):
            xts = [xpool.tile([P, Q], F32, name=f"xt{i}", tag="xt") for i in range(XB)]
            gs = [gpool.tile([P, TOPK, CW], F32, name=f"g{i}", tag="g") for i in range(GB)]
            gws = [wpool.tile([P, TOPK, E], BF16, name=f"gw{i}", tag="gw") for i in range(WB)]
            outfs = [opool.tile([P, E], F32, name=f"of{i}", tag="of") for i in range(OB)]
            iota_u = cpool.tile([P, 1], U32, name="iota_u")
            iota_f = cpool.tile([P, 1], F32, name="iota_f")

            # prologue: per-partition p*128 constant
            i_iota = pool(nc.gpsimd.iota(iota_u[:], [[1, 1]], channel_multiplier=P))
            dve(nc.vector.tensor_copy(iota_f[:], iota_u[:]), dep=i_iota)

            # per-tile state
            st = [dict() for _ in range(n_tiles)]

            def s_tile(i, shape, dt, nm):
                return spool.tile(shape, dt, name=f"{nm}{i}")

            NSEG = 4

            def emit_xload(i):
                # quarter-row loads: the first max8 segment starts once 1MB
                # (not 4MB) has landed -- trims ramp, smooths steady overlap
                xt = xts[i % XB]
                src_full = xd[i * P * NCH : (i + 1) * P * NCH, :].rearrange(
                    "(t c) w -> t (c w)", c=NCH
                )
                if 2 <= i < XB:
                    first = st[i - 2].get("m8") or st[i - 2].get("red")
                    if first is not None:
                        sp(nc.sync.nop(), dep=first)
                if i >= XB:
                    # WAR on xt rotation rides a gating NOP (HWDGE DMAs
                    # cannot carry manual waits: the ring-credit wait
                    # occupies their single wait slot)
                    sp(nc.sync.nop(), dep=st[i - XB]["xt_done"])
                qs = []
                w = Q // NSEG
                for k in range(NSEG):
                    d = sp(nc.sync.dma_start(
                        xt[:, k * w : (k + 1) * w], src_full[:, k * w : (k + 1) * w]
                    ))
                    dmas.append(d)
                    qs.append(d)
                st[i]["xload_q"] = qs
                st[i]["xload_a"] = qs[0]
                st[i]["xload"] = qs[-1]

            def emit_s1_b(i):
                """algo B stage 1: exact top-8 values, NSEG segments + merge
                (top-8 of per-segment top-8s equals the full top-8; segments
                start as soon as their quarter-load lands). Segment k's
                write-ack is covered by the following segments' execution;
                the merge's True-chain covers the last segment's ack."""
                xt = xts[i % XB]
                ab = s_tile(i, [P, 8 * NSEG], F32, "ab")
                v8 = s_tile(i, [P, 8], F32, "v8")
                w = Q // NSEG
                for k in range(NSEG):
                    dve(
                        nc.vector.max(
                            out=ab[:, 8 * k : 8 * (k + 1)],
                            in_=xt[:, k * w : (k + 1) * w],
                        ),
                        dep=st[i]["xload_q"][k],
                    )
                # ack-fence for the second-to-last segment's short cover
                dve(nc.vector.nop())
                st[i]["m8"] = dve(nc.vector.max(out=v8[:], in_=ab[:]))
                st[i]["v8_tile"] = v8

            def emit_s2_b(i):
                """algo B stage 2: indices via full-row find_index8."""
                xt = xts[i % XB]
                q8u = s_tile(i, [P, 8], U32, "q8u")
                st[i]["q8"] = dve(nc.vector.max_index(
                    out=q8u[:], in_max=st[i]["v8_tile"][:], in_values=xt[:],
                ))
                st[i]["xt_done"] = st[i]["q8"]
                st[i]["q8_tile"] = q8u

            def emit_s1(i):
                """chunk-max + chunk selection + gather-row math."""
                xt = xts[i % XB]
                c = s_tile(i, [P, NCH], F32, "c")
                cv8 = s_tile(i, [P, 8], F32, "cv8")
                ci8 = s_tile(i, [P, 8], U32, "ci8")
                ci8f = s_tile(i, [P, 8], F32, "ci8f")
                ci64f = s_tile(i, [P, 8], F32, "ci64f")
                row8f = s_tile(i, [P, 8], F32, "row8f")
                row8 = s_tile(i, [P, 8], U32, "row8")
                st[i]["red"] = dve(
                    nc.vector.tensor_reduce(
                        out=c[:],
                        in_=xt[:].rearrange("t (c w) -> t c w", w=CW),
                        axis=mybir.AxisListType.X,
                        op=Alu.max,
                    ),
                    dep=st[i]["xload"],
                )
                st[i]["xt_done"] = st[i]["red"]
                dve(nc.vector.max(out=cv8[:], in_=c[:]))
                dve(nc.vector.max_index(out=ci8[:], in_max=cv8[:], in_values=c[:]))
                dve(nc.vector.tensor_copy(ci8f[:], ci8[:]))
                dve(nc.vector.tensor_scalar(
                    out=ci64f[:], in0=ci8f[:], scalar1=64.0, scalar2=None,
                    op0=Alu.mult,
                ))
                dve(nc.vector.tensor_scalar(
                    out=row8f[:], in0=ci8f[:], scalar1=iota_f[:, 0:1],
                    scalar2=float(i * P * NCH), op0=Alu.add, op1=Alu.add,
                ))
                dve(nc.vector.tensor_scalar(
                    out=row8f[:], in0=row8f[:],
                    scalar1=float(t_core * NCH - 1), scalar2=0.0,
                    op0=Alu.min, op1=Alu.max,
                ))
                st[i]["row8"] = dve(nc.vector.tensor_copy(row8[:], row8f[:]))
                st[i]["row8_tile"] = row8
                st[i]["ci64f"] = ci64f
                if debug:
                    sl = slice(i * P, (i + 1) * P)
                    sp(nc.sync.nop(), dep=st[i]["row8"])
                    dmas.append(sp(nc.sync.dma_start(dbg_c[sl, :], c[:])))
                    dmas.append(sp(nc.sync.dma_start(dbg_ci8[sl, :], ci8[:])))
                    dmas.append(sp(nc.sync.dma_start(dbg_row8[sl, :], row8[:])))

            g_last_reader = {}   # buffer slot -> last instr reading it

            def emit_cgathers(i):
                g = gs[i % GB]
                pool(nc.gpsimd.nop(), dep=st[i]["row8"])
                prev_rd = g_last_reader.get(i % GB)
                if prev_rd is not None:
                    pool(nc.gpsimd.nop(), dep=prev_rd)
                cg = []
                r8 = st[i]["row8_tile"]
                for j in range(TOPK):
                    d = pool(nc.gpsimd.indirect_dma_start(
                        out=g[:, j, :], out_offset=None,
                        in_=xd[:],
                        in_offset=bass.IndirectOffsetOnAxis(
                            ap=r8[:, j : j + 1], axis=0
                        ),
                        compute_op=Alu.bypass,
                    ))
                    dmas.append(d)
                    cg.append(d)
                st[i]["cg"] = cg

            def emit_s2(i):
                """exact values + global index reconstruction."""
                g = gs[i % GB]
                v8 = s_tile(i, [P, 8], F32, "v8")
                local = s_tile(i, [P, 8], U32, "local")
                slot = s_tile(i, [P, 8], U32, "slot")
                slotf = s_tile(i, [P, 8], F32, "slotf")
                within = s_tile(i, [P, 8], U32, "within")
                withinf = s_tile(i, [P, 8], F32, "withinf")
                sel3 = s_tile(i, [P, 8, 8], F32, "sel3")
                csel = s_tile(i, [P, 8], F32, "csel")
                q8f = s_tile(i, [P, 8], F32, "q8f")
                q8u = s_tile(i, [P, 8], U32, "q8u")
                cg = st[i]["cg"]
                for j in range(TOPK - 1):
                    dve(nc.vector.nop(), dep=cg[j])
                dve(nc.vector.max(out=v8[:], in_=g[:]), dep=cg[TOPK - 1])
                st[i]["fi8g"] = dve(nc.vector.max_index(
                    out=local[:], in_max=v8[:],
                    in_values=g[:].rearrange("t s w -> t (s w)"),
                ))
                g_last_reader[i % GB] = st[i]["fi8g"]
                dve(nc.vector.tensor_scalar(
                    out=slot[:], in0=local[:], scalar1=6, scalar2=None,
                    op0=Alu.logical_shift_right,
                ))
                dve(nc.vector.tensor_scalar(
                    out=within[:], in0=local[:], scalar1=63, scalar2=None,
                    op0=Alu.bitwise_and,
                ))
                dve(nc.vector.tensor_copy(slotf[:], slot[:]))
                dve(nc.vector.tensor_copy(withinf[:], within[:]))
                ci64f = st[i]["ci64f"]
                for s in range(8):
                    dve(nc.vector.tensor_scalar(
                        out=sel3[:, :, s], in0=slotf[:], scalar1=float(s),
                        scalar2=ci64f[:, s : s + 1],
                        op0=Alu.is_equal, op1=Alu.mult,
                    ))
                dve(nc.vector.tensor_reduce(
                    out=csel[:], in_=sel3[:], axis=mybir.AxisListType.X,
                    op=Alu.add,
                ))
                dve(nc.vector.tensor_tensor(q8f[:], csel[:], withinf[:], Alu.add))
                dve(nc.vector.tensor_scalar(
                    out=q8f[:], in0=q8f[:],
                    scalar1=float(Q - 1), scalar2=0.0,
                    op0=Alu.min, op1=Alu.max,
                ))
                st[i]["q8"] = dve(nc.vector.tensor_copy(q8u[:], q8f[:]))
                st[i]["q8_tile"] = q8u
                if debug:
                    sl = slice(i * P, (i + 1) * P)
                    sp(nc.sync.nop(), dep=st[i]["q8"])
                    dmas.append(sp(nc.sync.dma_start(dbg_q8[sl, :], q8u[:])))
                    dmas.append(sp(nc.sync.dma_start(dbg_v8[sl, :], v8[:])))
                    dmas.append(sp(nc.sync.dma_start(dbg_g[sl, :, :], g[:])))

            gw_last_reader = {}

            def emit_wgathers(i):
                gw = gws[i % WB]
                pool(nc.gpsimd.nop(), dep=st[i]["q8"])
                prev_rd = gw_last_reader.get(i % WB)
                if prev_rd is not None:
                    pool(nc.gpsimd.nop(), dep=prev_rd)
                q8u = st[i]["q8_tile"]
                wg = []
                for j in range(TOPK):
                    d = pool(nc.gpsimd.indirect_dma_start(
                        out=gw[:, j, :], out_offset=None,
                        in_=wt[:],
                        in_offset=bass.IndirectOffsetOnAxis(
                            ap=q8u[:, j : j + 1], axis=0
                        ),
                        compute_op=Alu.bypass,
                    ))
                    dmas.append(d)
                    wg.append(d)
                st[i]["wg"] = wg

            def emit_s3(i):
                gw = gws[i % WB]
                outf = outfs[i % OB]
                wg = st[i]["wg"]
                for j in range(TOPK - 1):
                    dve(nc.vector.nop(), dep=wg[j])
                if i >= OB:
                    dve(nc.vector.nop(), dep=st[i - OB]["ostore"])
                dve(nc.vector.tensor_tensor(
                    gw[:, 0:4, :], gw[:, 0:4, :], gw[:, 4:8, :], Alu.add
                ), dep=wg[TOPK - 1])
                dve(nc.vector.tensor_tensor(
                    gw[:, 0:2, :], gw[:, 0:2, :], gw[:, 2:4, :], Alu.add
                ))
                st[i]["a3"] = dve(nc.vector.tensor_tensor(
                    outf[:], gw[:, 0, :], gw[:, 1, :], Alu.add
                ))
                gw_last_reader[i % WB] = st[i]["a3"]

            def emit_ostore(i):
                outf = outfs[i % OB]
                act(nc.scalar.nop(), dep=st[i]["a3"])
                d = act(nc.scalar.dma_start(out_d[i * P : (i + 1) * P, :], outf[:]))
                dmas.append(d)
                st[i]["ostore"] = d

            # software pipeline:
            #   DVE:  S1_i | S2_{i-1} | S3_{i-2}
            #   POOL: CG_i (after row8_i) | WG_{i-1} (after q8_{i-1})
            #   SP:   xload_{i+2} | ostore_{i-2}
            # prologue loads
            for i in range(min(2, n_tiles)):
                emit_xload(i)

            S2D = int(os.environ.get("S2D", "1"))
            for i in range(n_tiles + 4):
                if i < n_tiles:
                    if is_b[i]:
                        emit_s1_b(i)
                    else:
                        emit_s1(i)
                        emit_cgathers(i)
                if 0 <= i - S2D < n_tiles:
                    if is_b[i - S2D]:
                        emit_s2_b(i - S2D)
                    else:
                        emit_s2(i - S2D)
                    emit_wgathers(i - S2D)
                if i + 2 < n_tiles:
                    emit_xload(i + 2)
                if 0 <= i - 4 < n_tiles:
                    emit_s3(i - 4)
                    emit_ostore(i - 4)

            # tail quiesce
            for d in dmas:
                n = nc.sync.nop()
                strip(n)
                add_dep_helper(n.ins, d.ins, True, "tail quiesce")
            # observe the prologue iota + early DVE stream on SP so the
            # exit drain's waits are redundant and get elided
            for tgt in (i_iota, dve_q[0], dve_q[min(7, len(dve_q) - 1)]):
                n = nc.sync.nop()
                strip(n)
                add_dep_helper(n.ins, tgt.ins, True, "drain pre-observe")
            for lastq in (dve_q, pool_q, sp_q, act_q):
                n = nc.sync.nop()
                strip(n)
                add_dep_helper(n.ins, lastq[-1].ins, True, "tail quiesce")

    return nc


def _prep_wt(W: np.ndarray) -> np.ndarray:
    return np.ascontiguousarray(W.T).astype(ml_dtypes.bfloat16)


_CACHED = {}


def _get_nc():
    if "nc" not in _CACHED:
        _CACHED["nc"] = build_bass()
    return _CACHED["nc"]


def kernel(x: np.ndarray, W: np.ndarray) -> np.ndarray:
    x = np.asarray(x, dtype=np.float32)
    W = np.asarray(W, dtype=np.float32)
    assert x.shape == (B, S, Q) and W.shape == (E, Q)

    nc = _get_nc()
    xf = x.reshape(T_TOTAL, Q)
    WT = _prep_wt(W)
    in_maps = [
        {
            "x": np.ascontiguousarray(
                xf[c * T_CORE : (c + 1) * T_CORE]
            ).reshape(T_CORE * NCH, CW),
            "wt": WT,
        }
        for c in range(N_CORES)
    ]
    res = run_bass_kernel_spmd(nc, in_maps, core_ids=list(range(N_CORES)))
    out = np.concatenate([r["out"] for r in res.results], axis=0)
    return np.ascontiguousarray(out.reshape(B, S, E).astype(np.float32))
